# revision 7
# baseline (speedup 1.0000x reference)
"""Trainium2 Bass kernel for nn_AttentionAggregator (gnn_message_passing).

Strategy (8-way data parallel over nodes, per the sharding hint):
  - Algebraic reorder: relu(softmax(Q K^T) @ C @ W) == relu(softmax(Q K^T) @ (C @ W)),
    shrinking the dominant matmul from [4096,4096]@[4096,2048] to [4096,4096]@[4096,128].
  - Launch 1 (per core, 512 users + 512 items): device-side row gathers of the
    adjacency neighborhoods via the custom GPSIMD dma_gather:
      * item/user tables (4096 rows) gather directly with int16 indices;
      * the 100k-row review table is gathered in 4 <=25k-row windows (host
        buckets+pads the int16 indices) into a packed staging buffer, bounced
        through a DRAM scratch, then position-regathered (int16) into the
        original order -- all on device.
    Gathered tiles are PE-transposed and projected against host-restacked
    weight blocks (C @ W) -> h blocks [512,128] per side.
  - Host relays h (tiny) between launches.
  - Launch 2 (per core): S^T = K q^T on the tensor engine, exp on ScalarE with
    fused 1/sqrt(D) scale (scores ~N(0,1): max-subtraction unnecessary), PV
    matmul against [h | 1] (fused row-sum), reciprocal-normalize + relu.
"""

import sys

for _p in ("/opt/trn_rl_repo",):
    if _p not in sys.path:
        sys.path.append(_p)

import numpy as np

import concourse.bacc as bacc
import concourse.bass as bass
import concourse.mybir as mybir
import concourse.tile as tile
from concourse.bass_utils import run_bass_kernel_spmd
from concourse.masks import make_identity

F32 = mybir.dt.float32
I16 = mybir.dt.int16
AF = mybir.ActivationFunctionType

N_REV, NU, DEG, D, HID = 100000, 4096, 16, 64, 128
N_CORES = 8
UB = NU // N_CORES          # 512 rows per core per side
NT = UB // 128              # 4 user tiles per core
NSLOT = NT * DEG            # 64 gathered slots per side (c = t*16 + j)
NG = UB * DEG               # 8192 gathered rows per table per side
MT = NU // 128              # 32 m tiles
QB = UB
QT = QB // 128
G = 2                       # m-tiles per QK/exp/PV group in launch 2
NWIN = 4                    # review-table windows
WIN = (N_REV + NWIN - 1) // NWIN  # 25000 rows per window (< int16 max)
SCALE = 1.0 / float(np.sqrt(D))


def _build_k1(cap):
    assert cap % 128 == 0
    cslot = cap // 128
    nc = bacc.Bacc("TRN2", target_bir_lowering=False, debug=False,
                   enable_asserts=True, num_devices=N_CORES)
    rev = nc.dram_tensor("rev", [N_REV, D], F32, kind="ExternalInput")
    usert = nc.dram_tensor("usert", [NU, D], F32, kind="ExternalInput")
    itemt = nc.dram_tensor("itemt", [NU, D], F32, kind="ExternalInput")
    # host-restacked weight blocks: wa = stacked Wa_j rows, wb = stacked Wb_j
    wa_u = nc.dram_tensor("wa_u", [DEG * D, HID], F32, kind="ExternalInput")
    wb_u = nc.dram_tensor("wb_u", [DEG * D, HID], F32, kind="ExternalInput")
    wa_i = nc.dram_tensor("wa_i", [DEG * D, HID], F32, kind="ExternalInput")
    wb_i = nc.dram_tensor("wb_i", [DEG * D, HID], F32, kind="ExternalInput")
    # review gather: per side, 4 window index lists (padded to cap) + positions
    ridx = nc.dram_tensor("ridx", [2, NWIN, 128, cap // 16], I16, kind="ExternalInput")
    rpos = nc.dram_tensor("rpos", [2, 128, NG // 16], I16, kind="ExternalInput")
    # item/user gather indices per side
    iidx = nc.dram_tensor("iidx", [2, 128, NG // 16], I16, kind="ExternalInput")
    hu = nc.dram_tensor("hu", [UB, HID], F32, kind="ExternalOutput")
    hi = nc.dram_tensor("hi", [UB, HID], F32, kind="ExternalOutput")

    with tile.TileContext(nc) as tc:
        with (
            tc.tile_pool(name="singles", bufs=1) as singles,
            tc.tile_pool(name="stg", bufs=2) as stgp,
            tc.tile_pool(name="xr", bufs=2) as xrp,
            tc.tile_pool(name="xt", bufs=2) as xtp,
            tc.tile_pool(name="outb", bufs=4) as outb,
            tc.tile_pool(name="scr", bufs=2, space="DRAM") as scrp,
            tc.tile_pool(name="tps", bufs=4, space="PSUM") as tps,
            tc.tile_pool(name="hps", bufs=2, space="PSUM") as hps,
        ):
            ident = singles.tile([128, 128], F32)
            make_identity(nc, ident[:])
            w_sb = {}
            for nm, t in (("wa_u", wa_u), ("wb_u", wb_u), ("wa_i", wa_i), ("wb_i", wb_i)):
                w = singles.tile([128, DEG // 2, HID], F32, tag=nm, name=f"{nm}_sb")
                nc.sync.dma_start(out=w[:], in_=t.ap().rearrange("(q k) n -> k q n", q=DEG // 2))
                w_sb[nm] = w
            # partition-major index tiles
            ridx_sb = singles.tile([128, 2, NWIN, cap // 16], I16)
            nc.sync.dma_start(out=ridx_sb[:], in_=ridx.ap().rearrange("a w p s -> p a w s"))
            rpos_sb = singles.tile([128, 2, NG // 16], I16)
            nc.sync.dma_start(out=rpos_sb[:], in_=rpos.ap().rearrange("a p s -> p a s"))
            iidx_sb = singles.tile([128, 2, NG // 16], I16)
            nc.sync.dma_start(out=iidx_sb[:], in_=iidx.ap().rearrange("a p s -> p a s"))

            for side, (itbl, wa, wb, hout) in enumerate((
                (itemt, "wa_u", "wb_u", hu),
                (usert, "wa_i", "wb_i", hi),
            )):
                # stage 1: window gathers into packed staging
                stg = stgp.tile([128, NWIN * cslot, D], F32, tag="stg")
                for s in range(NWIN):
                    lo = s * WIN
                    hi_ = min(N_REV, lo + WIN)
                    nc.gpsimd.dma_gather(
                        out_ap=stg[:, s * cslot:(s + 1) * cslot, :],
                        in_ap=rev.ap()[lo:hi_, :],
                        idxs_ap=ridx_sb[:, side, s, :],
                        num_idxs=cap, num_idxs_reg=cap, elem_size=D, single_packet=False,
                    )
                # stage 2: staging -> DRAM scratch (row r = list position)
                scratch = scrp.tile([NWIN * cap, D], F32, tag="scr")
                for s in range(NWIN):
                    nc.sync.dma_start(
                        out=scratch[:].rearrange("(c p) e -> p c e", p=128)[:, s * cslot:(s + 1) * cslot, :],
                        in_=stg[:, s * cslot:(s + 1) * cslot, :],
                    )
                # stage 3: position re-gather into original (p, c) order
                XR = xrp.tile([128, NSLOT, D], F32, tag="XR")
                nc.gpsimd.dma_gather(
                    out_ap=XR[:], in_ap=scratch[:], idxs_ap=rpos_sb[:, side, :],
                    num_idxs=NG, num_idxs_reg=NG, elem_size=D, single_packet=False,
                )
                # item/user side gather (int16-native)
                XI = xrp.tile([128, NSLOT, D], F32, tag="XI")
                nc.gpsimd.dma_gather(
                    out_ap=XI[:], in_ap=itbl.ap(), idxs_ap=iidx_sb[:, side, :],
                    num_idxs=NG, num_idxs_reg=NG, elem_size=D, single_packet=False,
                )
                # per user tile: transpose slot pairs, project, accumulate
                for t in range(NT):
                    XT = xtp.tile([128, DEG, 128], F32, tag="XT")
                    for q in range(DEG // 2):
                        ps_r = tps.tile([128, 128], F32, tag="tps", name=f"tr{side}_{t}_{q}")
                        nc.tensor.transpose(ps_r[:], XR[:, t * DEG + 2 * q: t * DEG + 2 * q + 2, :], ident[:])
                        nc.vector.tensor_copy(XT[:, q, :], ps_r[:])
                        ps_i = tps.tile([128, 128], F32, tag="tps", name=f"ti{side}_{t}_{q}")
                        nc.tensor.transpose(ps_i[:], XI[:, t * DEG + 2 * q: t * DEG + 2 * q + 2, :], ident[:])
                        nc.vector.tensor_copy(XT[:, DEG // 2 + q, :], ps_i[:])
                    h_ps = hps.tile([128, HID], F32, tag="hps")
                    for q in range(DEG // 2):
                        nc.tensor.matmul(h_ps[:], lhsT=XT[:, q, :], rhs=w_sb[wa][:, q, :],
                                         start=(q == 0), stop=False)
                        nc.tensor.matmul(h_ps[:], lhsT=XT[:, DEG // 2 + q, :], rhs=w_sb[wb][:, q, :],
                                         start=False, stop=(q == DEG // 2 - 1))
                    h_sb = outb.tile([128, HID], F32, tag="hsb")
                    nc.vector.tensor_copy(h_sb[:], h_ps[:])
                    nc.sync.dma_start(out=hout.ap()[t * 128:(t + 1) * 128, :], in_=h_sb[:])
    nc.compile()
    return nc


def _build_k2():
    nc = bacc.Bacc("TRN2", target_bir_lowering=False, debug=False,
                   enable_asserts=True, num_devices=N_CORES)
    vtu = nc.dram_tensor("vtu", [D, NU], F32, kind="ExternalInput")
    vtuq = nc.dram_tensor("vtuq", [D, QB], F32, kind="ExternalInput")
    vti = nc.dram_tensor("vti", [D, NU], F32, kind="ExternalInput")
    vtiq = nc.dram_tensor("vtiq", [D, QB], F32, kind="ExternalInput")
    hau = nc.dram_tensor("hau", [128, MT, HID + 1], F32, kind="ExternalInput")
    hai = nc.dram_tensor("hai", [128, MT, HID + 1], F32, kind="ExternalInput")
    uo = nc.dram_tensor("uo", [QB, HID], F32, kind="ExternalOutput")
    io = nc.dram_tensor("io", [QB, HID], F32, kind="ExternalOutput")

    with tile.TileContext(nc) as tc:
        with (
            tc.tile_pool(name="vt", bufs=2) as vtp,
            tc.tile_pool(name="ha", bufs=2) as hap,
            tc.tile_pool(name="et", bufs=3) as etp,
            tc.tile_pool(name="ob", bufs=4) as obp,
            tc.tile_pool(name="sps", bufs=2, space="PSUM") as sps,
            tc.tile_pool(name="aps", bufs=1, space="PSUM") as aps,
        ):
            for side, (vt_d, vtq_d, ha_d, out_d) in enumerate((
                (vtu, vtuq, hau, uo), (vti, vtiq, hai, io),
            )):
                vt_sb = vtp.tile([D, NU], F32, tag="vt")
                vtq_sb = vtp.tile([D, QB], F32, tag="vtq")
                ha_sb = hap.tile([128, MT, HID + 1], F32, tag="ha")
                nc.sync.dma_start(out=vt_sb[:], in_=vt_d.ap())
                nc.sync.dma_start(out=vtq_sb[:], in_=vtq_d.ap())
                nc.sync.dma_start(out=ha_sb[:], in_=ha_d.ap())

                att_ps = [aps.tile([128, HID + 1], F32, tag=f"att{qt}", name=f"att{qt}_{side}")
                          for qt in range(QT)]
                for g in range(MT // G):
                    s_ps = sps.tile([128, G, QB], F32, tag="sps")
                    for k in range(G):
                        m = g * G + k
                        nc.tensor.matmul(
                            s_ps[:, k, :],
                            lhsT=vt_sb[:, m * 128:(m + 1) * 128],
                            rhs=vtq_sb[:],
                            start=True, stop=True,
                        )
                    et = etp.tile([128, G, QB], F32, tag="et")
                    nc.scalar.activation(et[:], s_ps[:], AF.Exp, scale=SCALE)
                    for k in range(G):
                        m = g * G + k
                        for qt in range(QT):
                            nc.tensor.matmul(
                                att_ps[qt][:],
                                lhsT=et[:, k, qt * 128:(qt + 1) * 128],
                                rhs=ha_sb[:, m, :],
                                start=(m == 0), stop=(m == MT - 1),
                                skip_group_check=True,
                            )
                for qt in range(QT):
                    recip = obp.tile([128, 1], F32, tag="recip")
                    nc.vector.reciprocal(recip[:], att_ps[qt][:, HID:HID + 1])
                    o_sb = obp.tile([128, HID], F32, tag="osb")
                    nc.scalar.activation(o_sb[:], att_ps[qt][:, 0:HID], AF.Relu,
                                         scale=recip[:, 0:1])
                    nc.sync.dma_start(out=out_d.ap()[qt * 128:(qt + 1) * 128, :], in_=o_sb[:])
    nc.compile()
    return nc


_CACHE = {}
DEFAULT_CAP = 3072


def _programs(cap):
    if ("k1", cap) not in _CACHE:
        _CACHE[("k1", cap)] = _build_k1(cap)
    if "k2" not in _CACHE:
        _CACHE["k2"] = _build_k2()
    return _CACHE[("k1", cap)], _CACHE["k2"]


def _arr(x, dt):
    return np.ascontiguousarray(np.asarray(x), dtype=dt)


def _wrap16(a):
    # flat int list -> [128, n/16] int16 buffer: index i at partition i%16,
    # slot i//16, replicated for the 8 Q7 cores
    a = np.asarray(a)
    return np.tile(a.reshape(-1, 16).T, (8, 1)).astype(np.int16)


def _cmajor(adj_blk):
    # [UB, DEG] -> flat vals[i], i = (t*DEG+j)*128 + p, user = t*128+p
    return adj_blk.reshape(NT, 128, DEG).transpose(0, 2, 1).reshape(-1)


def _prep_review(vals, cap):
    # bucket c-major list into NWIN windows; returns padded window index
    # lists [NWIN, cap] + positions [NG] into the packed scratch
    s = vals // WIN
    local = (vals - s * WIN).astype(np.int64)
    counts = np.bincount(s, minlength=NWIN)
    if counts.max() > cap:
        return None, None, int(counts.max())
    stage = np.zeros((NWIN, cap), np.int64)
    pos = np.empty(NG, np.int64)
    order = np.argsort(s, kind="stable")
    base = 0
    for w in range(NWIN):
        sel = order[base:base + counts[w]]
        stage[w, :counts[w]] = local[sel]
        pos[sel] = w * cap + np.arange(counts[w])
        base += counts[w]
    return stage, pos, int(counts.max())


def _stack_w(w):
    # [DEG*2*D, HID] -> (stacked Wa rows [DEG*D, HID], stacked Wb rows)
    w4 = w.reshape(DEG, 2, D, HID)
    wa = np.ascontiguousarray(w4[:, 0].reshape(DEG * D, HID))
    wb = np.ascontiguousarray(w4[:, 1].reshape(DEG * D, HID))
    return wa, wb


def _aug_tiled(h):
    ha = np.concatenate([h, np.ones((NU, 1), np.float32)], axis=1)
    return np.ascontiguousarray(ha.reshape(MT, 128, HID + 1).transpose(1, 0, 2))


def kernel(review_vecs, user_vecs, item_vecs, user_weights, item_weights,
           adj0, adj1, adj2, adj3, _profile=None):
    rev = _arr(review_vecs, np.float32)
    uv = _arr(user_vecs, np.float32)
    iv = _arr(item_vecs, np.float32)
    wu = _arr(user_weights, np.float32)
    wi = _arr(item_weights, np.float32)
    a0, a1, a2, a3 = (np.asarray(a).astype(np.int64) for a in (adj0, adj1, adj2, adj3))

    wa_u, wb_u = _stack_w(wu)
    wa_i, wb_i = _stack_w(wi)

    # host index prep; find a cap that fits all cores/sides
    cap = DEFAULT_CAP
    while True:
        ok = True
        preps = []
        for c in range(N_CORES):
            per_core = []
            for a_rev, a_oth in ((a0, a1), (a2, a3)):
                rvals = _cmajor(a_rev[c * UB:(c + 1) * UB])
                ovals = _cmajor(a_oth[c * UB:(c + 1) * UB])
                stage, pos, mx = _prep_review(rvals, cap)
                if stage is None:
                    ok = False
                    cap = ((mx + 127) // 128) * 128
                    break
                per_core.append((stage, pos, ovals))
            if not ok:
                break
            preps.append(per_core)
        if ok:
            break

    k1, k2 = _programs(cap)
    cores = list(range(N_CORES))

    in_maps1 = []
    for c in cores:
        ridx = np.zeros((2, NWIN, 128, cap // 16), np.int16)
        rpos = np.zeros((2, 128, NG // 16), np.int16)
        iidx = np.zeros((2, 128, NG // 16), np.int16)
        for side in range(2):
            stage, pos, ovals = preps[c][side]
            for w in range(NWIN):
                ridx[side, w] = _wrap16(stage[w])
            rpos[side] = _wrap16(pos)
            iidx[side] = _wrap16(ovals)
        in_maps1.append({
            "rev": rev, "usert": uv, "itemt": iv,
            "wa_u": wa_u, "wb_u": wb_u, "wa_i": wa_i, "wb_i": wb_i,
            "ridx": ridx, "rpos": rpos, "iidx": iidx,
        })
    r1 = run_bass_kernel_spmd(k1, in_maps1, core_ids=cores, trace=_profile is not None)
    h_user = np.concatenate([r1.results[c]["hu"] for c in cores], axis=0)
    h_item = np.concatenate([r1.results[c]["hi"] for c in cores], axis=0)

    uvt = np.ascontiguousarray(uv.T)
    ivt = np.ascontiguousarray(iv.T)
    hau = _aug_tiled(h_user)
    hai = _aug_tiled(h_item)
    in_maps2 = [{
        "vtu": uvt, "vtuq": np.ascontiguousarray(uvt[:, c * QB:(c + 1) * QB]),
        "vti": ivt, "vtiq": np.ascontiguousarray(ivt[:, c * QB:(c + 1) * QB]),
        "hau": hau, "hai": hai,
    } for c in cores]
    r2 = run_bass_kernel_spmd(k2, in_maps2, core_ids=cores, trace=_profile is not None)

    user_out = np.concatenate([r2.results[c]["uo"] for c in cores], axis=0)
    item_out = np.concatenate([r2.results[c]["io"] for c in cores], axis=0)

    if _profile is not None:
        _profile["k1"] = r1
        _profile["k2"] = r2
    return user_out, item_out


# revision 9
# speedup vs baseline: 1.4196x; 1.4196x over previous
"""Trainium2 Bass kernel for nn_AttentionAggregator (gnn_message_passing).

Strategy (8-way data parallel over nodes, per the sharding hint):
  - Algebraic reorder: relu(softmax(Q K^T) @ C @ W) == relu(softmax(Q K^T) @ (C @ W)),
    shrinking the dominant matmul from [4096,4096]@[4096,2048] to [4096,4096]@[4096,128].
  - The serial cost floor on-device is GPSIMD descriptor generation for the
    data-dependent row gathers (~8 ns/row).  Launch 1 therefore runs the
    gathers on GPSIMD while *all* other engines compute underneath that
    shadow:
      * review rows arrive as 1KB 4-row blocks via the custom dma_gather
        (block id r//4 fits int16 with no windowing/sorting); a 4-way
        masked DVE select picks row r%4 per entry;
      * item/user rows (4096-row tables) gather directly with int16 ids;
      * gathered tiles are PE-transposed in slot pairs and projected against
        host-restacked weight blocks -> h blocks [512,128] per side;
      * concurrently the dense scores S^T = K q^T run on the tensor engine and
        exp(S/8) on ScalarE (scores ~N(0,1): no max subtraction), emitting
        E^T in bf16 to DRAM.
  - Host relays h (tiny) and E^T between launches.
  - Launch 2 (per core): PV matmul (bf16 E^T stationary -> fast weight load)
    against [h | 1] bf16 (fused row-sum), reciprocal-normalize + relu.
"""

import sys

for _p in ("/opt/trn_rl_repo",):
    if _p not in sys.path:
        sys.path.append(_p)

import numpy as np

import concourse.bacc as bacc
import concourse.bass as bass
import concourse.mybir as mybir
import concourse.tile as tile
from concourse.bass_utils import run_bass_kernel_spmd
from concourse.masks import make_identity

F32 = mybir.dt.float32
BF16 = mybir.dt.bfloat16
I16 = mybir.dt.int16
AF = mybir.ActivationFunctionType
MULT = mybir.AluOpType.mult
ADD = mybir.AluOpType.add

N_REV, NU, DEG, D, HID = 100000, 4096, 16, 64, 128
N_CORES = 8
UB = NU // N_CORES          # 512 rows per core per side
NT = UB // 128              # 4 user tiles per core
NSLOT = NT * DEG            # 64 gathered slots per side (c = t*16 + j)
NG = UB * DEG               # 8192 gathered rows per table per side
MT = NU // 128              # 32 m tiles
QB = UB
QT = QB // 128
G = 2                       # m-tiles per QK/exp group
BLK = 4                     # review rows per gathered block
SCALE = 1.0 / float(np.sqrt(D))


def _build_k1():
    nc = bacc.Bacc("TRN2", target_bir_lowering=False, debug=False,
                   enable_asserts=True, num_devices=N_CORES)
    rev = nc.dram_tensor("rev", [N_REV, D], F32, kind="ExternalInput")
    usert = nc.dram_tensor("usert", [NU, D], F32, kind="ExternalInput")
    itemt = nc.dram_tensor("itemt", [NU, D], F32, kind="ExternalInput")
    wa_u = nc.dram_tensor("wa_u", [DEG * D, HID], F32, kind="ExternalInput")
    wb_u = nc.dram_tensor("wb_u", [DEG * D, HID], F32, kind="ExternalInput")
    wa_i = nc.dram_tensor("wa_i", [DEG * D, HID], F32, kind="ExternalInput")
    wb_i = nc.dram_tensor("wb_i", [DEG * D, HID], F32, kind="ExternalInput")
    bidx = nc.dram_tensor("bidx", [2, 128, NG // 16], I16, kind="ExternalInput")
    selm = nc.dram_tensor("selm", [2, 128, NSLOT, BLK], F32, kind="ExternalInput")
    iidx = nc.dram_tensor("iidx", [2, 128, NG // 16], I16, kind="ExternalInput")
    vtu = nc.dram_tensor("vtu", [D, NU], F32, kind="ExternalInput")
    vtuq = nc.dram_tensor("vtuq", [D, QB], F32, kind="ExternalInput")
    vti = nc.dram_tensor("vti", [D, NU], F32, kind="ExternalInput")
    vtiq = nc.dram_tensor("vtiq", [D, QB], F32, kind="ExternalInput")
    hu = nc.dram_tensor("hu", [UB, HID], F32, kind="ExternalOutput")
    hi = nc.dram_tensor("hi", [UB, HID], F32, kind="ExternalOutput")
    et = nc.dram_tensor("et", [2, MT, 128, QB], BF16, kind="ExternalOutput")

    with tile.TileContext(nc) as tc:
        with (
            tc.tile_pool(name="singles", bufs=1) as singles,
            tc.tile_pool(name="stgp", bufs=1) as stgp,
            tc.tile_pool(name="xp", bufs=1) as xp,
            tc.tile_pool(name="xtp", bufs=2) as xtp,
            tc.tile_pool(name="outb", bufs=4) as outb,
            tc.tile_pool(name="vtp", bufs=1) as vtp,
            tc.tile_pool(name="etp", bufs=3) as etp,
            tc.tile_pool(name="sps", bufs=2, space="PSUM") as sps,
            tc.tile_pool(name="tps", bufs=2, space="PSUM") as tps,
            tc.tile_pool(name="hps", bufs=2, space="PSUM") as hps,
        ):
            ident = singles.tile([128, 128], F32)
            make_identity(nc, ident[:])
            w_sb = {}
            for nm, t in (("wa_u", wa_u), ("wb_u", wb_u), ("wa_i", wa_i), ("wb_i", wb_i)):
                w = singles.tile([128, DEG // 2, HID], F32, tag=nm, name=f"{nm}_sb")
                nc.sync.dma_start(out=w[:], in_=t.ap().rearrange("(q k) n -> k q n", q=DEG // 2))
                w_sb[nm] = w
            bidx_sb = singles.tile([128, 2, NG // 16], I16)
            nc.sync.dma_start(out=bidx_sb[:], in_=bidx.ap().rearrange("a p s -> p a s"))
            iidx_sb = singles.tile([128, 2, NG // 16], I16)
            nc.sync.dma_start(out=iidx_sb[:], in_=iidx.ap().rearrange("a p s -> p a s"))
            selm_sb = singles.tile([128, 2, NSLOT, BLK], F32)
            nc.sync.dma_start(out=selm_sb[:], in_=selm.ap().rearrange("a p c b -> p a c b"))

            for side, (itbl, vt_d, vtq_d, wa, wb, hout) in enumerate((
                (itemt, vtu, vtuq, "wa_u", "wb_u", hu),
                (usert, vti, vtiq, "wa_i", "wb_i", hi),
            )):
                # ---- dense scores + exp (runs under the gather shadow) ----
                vt_sb = vtp.tile([D, NU], F32, tag="vt")
                vtq_sb = vtp.tile([D, QB], F32, tag="vtq")
                nc.sync.dma_start(out=vt_sb[:], in_=vt_d.ap())
                nc.sync.dma_start(out=vtq_sb[:], in_=vtq_d.ap())
                for g in range(MT // G):
                    s_ps = sps.tile([128, G, QB], F32, tag="sps")
                    for k in range(G):
                        m = g * G + k
                        nc.tensor.matmul(
                            s_ps[:, k, :],
                            lhsT=vt_sb[:, m * 128:(m + 1) * 128],
                            rhs=vtq_sb[:],
                            start=True, stop=True,
                        )
                    etb = etp.tile([128, G, QB], BF16, tag="etb")
                    nc.scalar.activation(etb[:], s_ps[:], AF.Exp, scale=SCALE)
                    nc.sync.dma_start(
                        out=et.ap()[side, g * G:(g + 1) * G, :, :].rearrange("g p q -> p g q"),
                        in_=etb[:],
                    )

                # ---- review gather: 1KB 4-row blocks + masked DVE select ----
                stg = stgp.tile([128, NSLOT, BLK * D], F32, tag="stg")
                nc.gpsimd.dma_gather(
                    out_ap=stg[:], in_ap=rev.ap().rearrange("(n b) e -> n (b e)", b=BLK),
                    idxs_ap=bidx_sb[:, side, :],
                    num_idxs=NG, num_idxs_reg=NG, elem_size=BLK * D,
                    single_packet=False,
                )
                XR = xp.tile([128, NSLOT, D], F32, tag="XR")
                tmp = xp.tile([128, NSLOT, D], F32, tag="tmp")
                for b in range(BLK):
                    mb = selm_sb[:, side, :, b][:, :, None].broadcast_to([128, NSLOT, D])
                    dst = XR if b == 0 else tmp
                    nc.vector.tensor_tensor(out=dst[:], in0=stg[:, :, b * D:(b + 1) * D], in1=mb, op=MULT)
                    if b > 0:
                        nc.vector.tensor_tensor(out=XR[:], in0=XR[:], in1=tmp[:], op=ADD)

                # ---- item/user gather (int16-native) ----
                XI = xp.tile([128, NSLOT, D], F32, tag="XI")
                nc.gpsimd.dma_gather(
                    out_ap=XI[:], in_ap=itbl.ap(), idxs_ap=iidx_sb[:, side, :],
                    num_idxs=NG, num_idxs_reg=NG, elem_size=D,
                    single_packet=False,
                )

                # ---- transpose slot pairs + project -> h ----
                for t in range(NT):
                    XT = xtp.tile([128, DEG, 128], F32, tag="XT")
                    for q in range(DEG // 2):
                        ps_r = tps.tile([128, 128], F32, tag="tps", name=f"tr{side}_{t}_{q}")
                        nc.tensor.transpose(ps_r[:], XR[:, t * DEG + 2 * q: t * DEG + 2 * q + 2, :], ident[:])
                        nc.vector.tensor_copy(XT[:, q, :], ps_r[:])
                        ps_i = tps.tile([128, 128], F32, tag="tps", name=f"ti{side}_{t}_{q}")
                        nc.tensor.transpose(ps_i[:], XI[:, t * DEG + 2 * q: t * DEG + 2 * q + 2, :], ident[:])
                        nc.vector.tensor_copy(XT[:, DEG // 2 + q, :], ps_i[:])
                    h_ps = hps.tile([128, HID], F32, tag="hps")
                    for q in range(DEG // 2):
                        nc.tensor.matmul(h_ps[:], lhsT=XT[:, q, :], rhs=w_sb[wa][:, q, :],
                                         start=(q == 0), stop=False, skip_group_check=True)
                        nc.tensor.matmul(h_ps[:], lhsT=XT[:, DEG // 2 + q, :], rhs=w_sb[wb][:, q, :],
                                         start=False, stop=(q == DEG // 2 - 1), skip_group_check=True)
                    h_sb = outb.tile([128, HID], F32, tag="hsb")
                    nc.vector.tensor_copy(h_sb[:], h_ps[:])
                    nc.sync.dma_start(out=hout.ap()[t * 128:(t + 1) * 128, :], in_=h_sb[:])
    nc.compile()
    return nc


def _build_k2():
    nc = bacc.Bacc("TRN2", target_bir_lowering=False, debug=False,
                   enable_asserts=True, num_devices=N_CORES)
    et = nc.dram_tensor("et", [2, MT, 128, QB], BF16, kind="ExternalInput")
    hau = nc.dram_tensor("hau", [128, MT, HID + 1], BF16, kind="ExternalInput")
    hai = nc.dram_tensor("hai", [128, MT, HID + 1], BF16, kind="ExternalInput")
    uo = nc.dram_tensor("uo", [QB, HID], F32, kind="ExternalOutput")
    io = nc.dram_tensor("io", [QB, HID], F32, kind="ExternalOutput")

    with tile.TileContext(nc) as tc:
        with (
            tc.tile_pool(name="etp", bufs=2) as etp,
            tc.tile_pool(name="ha", bufs=2) as hap,
            tc.tile_pool(name="ob", bufs=4) as obp,
            tc.tile_pool(name="aps", bufs=1, space="PSUM") as aps,
        ):
            for side, (ha_d, out_d) in enumerate(((hau, uo), (hai, io))):
                et_sb = etp.tile([128, MT, QB], BF16, tag="et")
                nc.sync.dma_start(out=et_sb[:], in_=et.ap()[side].rearrange("m p q -> p m q"))
                ha_sb = hap.tile([128, MT, HID + 1], BF16, tag="ha")
                nc.sync.dma_start(out=ha_sb[:], in_=ha_d.ap())

                att_ps = [aps.tile([128, HID + 1], F32, tag=f"att{qt}", name=f"att{qt}_{side}")
                          for qt in range(QT)]
                for m in range(MT):
                    for qt in range(QT):
                        nc.tensor.matmul(
                            att_ps[qt][:],
                            lhsT=et_sb[:, m, qt * 128:(qt + 1) * 128],
                            rhs=ha_sb[:, m, :],
                            start=(m == 0), stop=(m == MT - 1),
                            skip_group_check=True,
                        )
                for qt in range(QT):
                    recip = obp.tile([128, 1], F32, tag="recip")
                    nc.vector.reciprocal(recip[:], att_ps[qt][:, HID:HID + 1])
                    o_sb = obp.tile([128, HID], F32, tag="osb")
                    nc.scalar.activation(o_sb[:], att_ps[qt][:, 0:HID], AF.Relu,
                                         scale=recip[:, 0:1])
                    nc.sync.dma_start(out=out_d.ap()[qt * 128:(qt + 1) * 128, :], in_=o_sb[:])
    nc.compile()
    return nc


_CACHE = {}


def _programs():
    if "k1" not in _CACHE:
        _CACHE["k1"] = _build_k1()
        _CACHE["k2"] = _build_k2()
    return _CACHE["k1"], _CACHE["k2"]


def _arr(x, dt):
    return np.ascontiguousarray(np.asarray(x), dtype=dt)


def _wrap16(a):
    # flat int list -> [128, n/16] int16: index i at partition i%16, slot
    # i//16, replicated for the 8 Q7 cores
    a = np.asarray(a)
    return np.tile(a.reshape(-1, 16).T, (8, 1)).astype(np.int16)


def _cmajor(adj_blk):
    # [UB, DEG] -> flat vals[i], i = (t*DEG+j)*128 + p, user = t*128+p
    return adj_blk.reshape(NT, 128, DEG).transpose(0, 2, 1).reshape(-1)


def _stack_w(w):
    w4 = w.reshape(DEG, 2, D, HID)
    wa = np.ascontiguousarray(w4[:, 0].reshape(DEG * D, HID))
    wb = np.ascontiguousarray(w4[:, 1].reshape(DEG * D, HID))
    return wa, wb


def _aug_tiled(h):
    import ml_dtypes
    ha = np.concatenate([h, np.ones((NU, 1), np.float32)], axis=1)
    ha = ha.reshape(MT, 128, HID + 1).transpose(1, 0, 2)
    return np.ascontiguousarray(ha.astype(ml_dtypes.bfloat16))


def kernel(review_vecs, user_vecs, item_vecs, user_weights, item_weights,
           adj0, adj1, adj2, adj3, _profile=None):
    rev = _arr(review_vecs, np.float32)
    uv = _arr(user_vecs, np.float32)
    iv = _arr(item_vecs, np.float32)
    wu = _arr(user_weights, np.float32)
    wi = _arr(item_weights, np.float32)
    a0, a1, a2, a3 = (np.asarray(a).astype(np.int64) for a in (adj0, adj1, adj2, adj3))

    wa_u, wb_u = _stack_w(wu)
    wa_i, wb_i = _stack_w(wi)
    uvt = np.ascontiguousarray(uv.T)
    ivt = np.ascontiguousarray(iv.T)

    k1, k2 = _programs()
    cores = list(range(N_CORES))

    in_maps1 = []
    for c in cores:
        bidx = np.zeros((2, 128, NG // 16), np.int16)
        iidx = np.zeros((2, 128, NG // 16), np.int16)
        selm = np.zeros((2, 128, NSLOT, BLK), np.float32)
        for side, (a_rev, a_oth) in enumerate(((a0, a1), (a2, a3))):
            rvals = _cmajor(a_rev[c * UB:(c + 1) * UB])
            ovals = _cmajor(a_oth[c * UB:(c + 1) * UB])
            bidx[side] = _wrap16(rvals // BLK)
            iidx[side] = _wrap16(ovals)
            sel = (rvals % BLK).reshape(NSLOT, 128).T  # [p, c]
            for b in range(BLK):
                selm[side, :, :, b] = (sel == b)
        in_maps1.append({
            "rev": rev, "usert": uv, "itemt": iv,
            "wa_u": wa_u, "wb_u": wb_u, "wa_i": wa_i, "wb_i": wb_i,
            "bidx": bidx, "selm": selm, "iidx": iidx,
            "vtu": uvt, "vtuq": np.ascontiguousarray(uvt[:, c * QB:(c + 1) * QB]),
            "vti": ivt, "vtiq": np.ascontiguousarray(ivt[:, c * QB:(c + 1) * QB]),
        })
    r1 = run_bass_kernel_spmd(k1, in_maps1, core_ids=cores, trace=_profile is not None)
    h_user = np.concatenate([r1.results[c]["hu"] for c in cores], axis=0)
    h_item = np.concatenate([r1.results[c]["hi"] for c in cores], axis=0)

    hau = _aug_tiled(h_user)
    hai = _aug_tiled(h_item)
    in_maps2 = [{
        "et": r1.results[c]["et"], "hau": hau, "hai": hai,
    } for c in cores]
    r2 = run_bass_kernel_spmd(k2, in_maps2, core_ids=cores, trace=_profile is not None)

    user_out = np.concatenate([r2.results[c]["uo"] for c in cores], axis=0)
    item_out = np.concatenate([r2.results[c]["io"] for c in cores], axis=0)

    if _profile is not None:
        _profile["k1"] = r1
        _profile["k2"] = r2
    return user_out, item_out


# revision 10
# speedup vs baseline: 1.7483x; 1.2315x over previous
"""Trainium2 Bass kernel for nn_AttentionAggregator (gnn_message_passing).

Strategy (8-way data parallel over nodes, per the sharding hint):
  - Algebraic reorder: relu(softmax(Q K^T) @ C @ W) == relu(softmax(Q K^T) @ (C @ W)),
    shrinking the dominant matmul from [4096,4096]@[4096,2048] to [4096,4096]@[4096,128].
  - The serial cost floor on-device is GPSIMD descriptor generation for the
    data-dependent row gathers (~8 ns/row).  Launch 1 therefore runs the
    gathers on GPSIMD while *all* other engines compute underneath that
    shadow:
      * review rows arrive as 1KB 4-row blocks via the custom dma_gather
        (block id r//4 fits int16 with no windowing/sorting); a 4-way
        masked DVE select picks row r%4 per entry;
      * item/user rows (4096-row tables) gather directly with int16 ids;
      * gathered tiles are PE-transposed in slot pairs and projected against
        host-restacked weight blocks -> h blocks [512,128] per side;
      * concurrently the dense scores S^T = K q^T run on the tensor engine and
        exp(S/8) on ScalarE (scores ~N(0,1): no max subtraction), emitting
        E^T in bf16 to DRAM.
  - Host relays h (tiny) and E^T between launches.
  - Launch 2 (per core): PV matmul (bf16 E^T stationary -> fast weight load)
    against [h | 1] bf16 (fused row-sum), reciprocal-normalize + relu.
"""

import sys

for _p in ("/opt/trn_rl_repo",):
    if _p not in sys.path:
        sys.path.append(_p)

import numpy as np

import concourse.bacc as bacc
import concourse.bass as bass
import concourse.mybir as mybir
import concourse.tile as tile
from concourse.bass_utils import run_bass_kernel_spmd
from concourse.masks import make_identity

F32 = mybir.dt.float32
BF16 = mybir.dt.bfloat16
I16 = mybir.dt.int16
AF = mybir.ActivationFunctionType
MULT = mybir.AluOpType.mult
ADD = mybir.AluOpType.add

N_REV, NU, DEG, D, HID = 100000, 4096, 16, 64, 128
N_CORES = 8
UB = NU // N_CORES          # 512 rows per core per side
NT = UB // 128              # 4 user tiles per core
NSLOT = NT * DEG            # 64 gathered slots per side (c = t*16 + j)
NG = UB * DEG               # 8192 gathered rows per table per side
MT = NU // 128              # 32 m tiles
QB = UB
QT = QB // 128
G = 2                       # m-tiles per QK/exp group
BLK = 4                     # review rows per gathered block
SCALE = 1.0 / float(np.sqrt(D))


def _build_k1():
    nc = bacc.Bacc("TRN2", target_bir_lowering=False, debug=False,
                   enable_asserts=True, num_devices=N_CORES,
                   num_swdge_queues=4)
    rev = nc.dram_tensor("rev", [N_REV, D], F32, kind="ExternalInput")
    usert = nc.dram_tensor("usert", [NU, D], F32, kind="ExternalInput")
    itemt = nc.dram_tensor("itemt", [NU, D], F32, kind="ExternalInput")
    wa_u = nc.dram_tensor("wa_u", [DEG * D, HID], F32, kind="ExternalInput")
    wb_u = nc.dram_tensor("wb_u", [DEG * D, HID], F32, kind="ExternalInput")
    wa_i = nc.dram_tensor("wa_i", [DEG * D, HID], F32, kind="ExternalInput")
    wb_i = nc.dram_tensor("wb_i", [DEG * D, HID], F32, kind="ExternalInput")
    bidx = nc.dram_tensor("bidx", [2, 128, NG // 16], I16, kind="ExternalInput")
    selm = nc.dram_tensor("selm", [2, 128, NSLOT, BLK], F32, kind="ExternalInput")
    iidx = nc.dram_tensor("iidx", [2, 128, NG // 16], I16, kind="ExternalInput")
    vtu = nc.dram_tensor("vtu", [D, NU], BF16, kind="ExternalInput")
    vtuq = nc.dram_tensor("vtuq", [D, QB], BF16, kind="ExternalInput")
    vti = nc.dram_tensor("vti", [D, NU], BF16, kind="ExternalInput")
    vtiq = nc.dram_tensor("vtiq", [D, QB], BF16, kind="ExternalInput")
    hu = nc.dram_tensor("hu", [UB, HID], F32, kind="ExternalOutput")
    hi = nc.dram_tensor("hi", [UB, HID], F32, kind="ExternalOutput")
    et = nc.dram_tensor("et", [2, MT, 128, QB], BF16, kind="ExternalOutput")

    with tile.TileContext(nc) as tc:
        with (
            tc.tile_pool(name="singles", bufs=1) as singles,
            tc.tile_pool(name="stgp", bufs=1) as stgp,
            tc.tile_pool(name="xp", bufs=1) as xp,
            tc.tile_pool(name="xtp", bufs=2) as xtp,
            tc.tile_pool(name="outb", bufs=4) as outb,
            tc.tile_pool(name="vtp", bufs=1) as vtp,
            tc.tile_pool(name="etp", bufs=3) as etp,
            tc.tile_pool(name="sps", bufs=2, space="PSUM") as sps,
            tc.tile_pool(name="tps", bufs=2, space="PSUM") as tps,
            tc.tile_pool(name="hps", bufs=2, space="PSUM") as hps,
        ):
            ident = singles.tile([128, 128], F32)
            make_identity(nc, ident[:])
            w_sb = {}
            for nm, t in (("wa_u", wa_u), ("wb_u", wb_u), ("wa_i", wa_i), ("wb_i", wb_i)):
                w = singles.tile([128, DEG // 2, HID], F32, tag=nm, name=f"{nm}_sb")
                nc.sync.dma_start(out=w[:], in_=t.ap().rearrange("(q k) n -> k q n", q=DEG // 2))
                w_sb[nm] = w
            bidx_sb = singles.tile([128, 2, NG // 16], I16)
            nc.sync.dma_start(out=bidx_sb[:], in_=bidx.ap().rearrange("a p s -> p a s"))
            iidx_sb = singles.tile([128, 2, NG // 16], I16)
            nc.sync.dma_start(out=iidx_sb[:], in_=iidx.ap().rearrange("a p s -> p a s"))
            selm_sb = singles.tile([128, 2, NSLOT, BLK], F32)
            nc.sync.dma_start(out=selm_sb[:], in_=selm.ap().rearrange("a p c b -> p a c b"))

            for side, (itbl, vt_d, vtq_d, wa, wb, hout) in enumerate((
                (itemt, vtu, vtuq, "wa_u", "wb_u", hu),
                (usert, vti, vtiq, "wa_i", "wb_i", hi),
            )):
                # ---- dense scores + exp (runs under the gather shadow) ----
                vt_sb = vtp.tile([D, NU], BF16, tag="vt")
                vtq_sb = vtp.tile([D, QB], BF16, tag="vtq")
                nc.sync.dma_start(out=vt_sb[:], in_=vt_d.ap())
                nc.sync.dma_start(out=vtq_sb[:], in_=vtq_d.ap())
                for g in range(MT // G):
                    s_ps = sps.tile([128, G, QB], F32, tag="sps")
                    for k in range(G):
                        m = g * G + k
                        nc.tensor.matmul(
                            s_ps[:, k, :],
                            lhsT=vt_sb[:, m * 128:(m + 1) * 128],
                            rhs=vtq_sb[:],
                            start=True, stop=True,
                        )
                    etb = etp.tile([128, G, QB], BF16, tag="etb")
                    nc.scalar.activation(etb[:], s_ps[:], AF.Exp, scale=SCALE)
                    nc.sync.dma_start(
                        out=et.ap()[side, g * G:(g + 1) * G, :, :].rearrange("g p q -> p g q"),
                        in_=etb[:],
                    )

                # ---- gathers in half-slot chunks, rotated across SWDGE
                # queues so descriptor generation never stalls on ring drain
                stg = stgp.tile([128, NSLOT, BLK * D], F32, tag="stg")
                XR = xp.tile([128, NSLOT, D], F32, tag="XR")
                tmp = xp.tile([128, NSLOT, D], F32, tag="tmp")
                XI = xp.tile([128, NSLOT, D], F32, tag="XI")
                HS = NSLOT // 2      # 32 slots per half
                HG = NG // 2         # 4096 rows per half
                for h in range(2):
                    sl = slice(h * HS, (h + 1) * HS)
                    iw = slice(h * (HG // 16), (h + 1) * (HG // 16))
                    q0 = (side * 2 + h) % 4
                    nc.gpsimd.dma_gather(
                        out_ap=stg[:, sl, :],
                        in_ap=rev.ap().rearrange("(n b) e -> n (b e)", b=BLK),
                        idxs_ap=bidx_sb[:, side, iw],
                        num_idxs=HG, num_idxs_reg=HG, elem_size=BLK * D,
                        single_packet=False, queue_num=q0,
                    )
                    nc.gpsimd.dma_gather(
                        out_ap=XI[:, sl, :], in_ap=itbl.ap(),
                        idxs_ap=iidx_sb[:, side, iw],
                        num_idxs=HG, num_idxs_reg=HG, elem_size=D,
                        single_packet=False, queue_num=(q0 + 1) % 4,
                    )
                    for b in range(BLK):
                        mb = selm_sb[:, side, sl, b][:, :, None].broadcast_to([128, HS, D])
                        dst = XR if b == 0 else tmp
                        nc.vector.tensor_tensor(out=dst[:, sl, :], in0=stg[:, sl, b * D:(b + 1) * D],
                                                in1=mb, op=MULT)
                        if b > 0:
                            nc.vector.tensor_tensor(out=XR[:, sl, :], in0=XR[:, sl, :],
                                                    in1=tmp[:, sl, :], op=ADD)

                # ---- transpose slot pairs + project -> h ----
                for t in range(NT):
                    XT = xtp.tile([128, DEG, 128], F32, tag="XT")
                    for q in range(DEG // 2):
                        ps_r = tps.tile([128, 128], F32, tag="tps", name=f"tr{side}_{t}_{q}")
                        nc.tensor.transpose(ps_r[:], XR[:, t * DEG + 2 * q: t * DEG + 2 * q + 2, :], ident[:])
                        nc.vector.tensor_copy(XT[:, q, :], ps_r[:])
                        ps_i = tps.tile([128, 128], F32, tag="tps", name=f"ti{side}_{t}_{q}")
                        nc.tensor.transpose(ps_i[:], XI[:, t * DEG + 2 * q: t * DEG + 2 * q + 2, :], ident[:])
                        nc.vector.tensor_copy(XT[:, DEG // 2 + q, :], ps_i[:])
                    h_ps = hps.tile([128, HID], F32, tag="hps")
                    for q in range(DEG // 2):
                        nc.tensor.matmul(h_ps[:], lhsT=XT[:, q, :], rhs=w_sb[wa][:, q, :],
                                         start=(q == 0), stop=False, skip_group_check=True)
                        nc.tensor.matmul(h_ps[:], lhsT=XT[:, DEG // 2 + q, :], rhs=w_sb[wb][:, q, :],
                                         start=False, stop=(q == DEG // 2 - 1), skip_group_check=True)
                    h_sb = outb.tile([128, HID], F32, tag="hsb")
                    nc.vector.tensor_copy(h_sb[:], h_ps[:])
                    nc.sync.dma_start(out=hout.ap()[t * 128:(t + 1) * 128, :], in_=h_sb[:])
    nc.compile()
    return nc


def _build_k2():
    nc = bacc.Bacc("TRN2", target_bir_lowering=False, debug=False,
                   enable_asserts=True, num_devices=N_CORES)
    et = nc.dram_tensor("et", [2, MT, 128, QB], BF16, kind="ExternalInput")
    hau = nc.dram_tensor("hau", [128, MT, HID + 1], BF16, kind="ExternalInput")
    hai = nc.dram_tensor("hai", [128, MT, HID + 1], BF16, kind="ExternalInput")
    uo = nc.dram_tensor("uo", [QB, HID], F32, kind="ExternalOutput")
    io = nc.dram_tensor("io", [QB, HID], F32, kind="ExternalOutput")

    with tile.TileContext(nc) as tc:
        with (
            tc.tile_pool(name="etp", bufs=2) as etp,
            tc.tile_pool(name="ha", bufs=2) as hap,
            tc.tile_pool(name="ob", bufs=4) as obp,
            tc.tile_pool(name="aps", bufs=1, space="PSUM") as aps,
        ):
            for side, (ha_d, out_d) in enumerate(((hau, uo), (hai, io))):
                et_sb = etp.tile([128, MT, QB], BF16, tag="et")
                nc.sync.dma_start(out=et_sb[:], in_=et.ap()[side].rearrange("m p q -> p m q"))
                ha_sb = hap.tile([128, MT, HID + 1], BF16, tag="ha")
                nc.sync.dma_start(out=ha_sb[:], in_=ha_d.ap())

                att_ps = [aps.tile([128, HID + 1], F32, tag=f"att{qt}", name=f"att{qt}_{side}")
                          for qt in range(QT)]
                for m in range(MT):
                    for qt in range(QT):
                        nc.tensor.matmul(
                            att_ps[qt][:],
                            lhsT=et_sb[:, m, qt * 128:(qt + 1) * 128],
                            rhs=ha_sb[:, m, :],
                            start=(m == 0), stop=(m == MT - 1),
                            skip_group_check=True,
                        )
                for qt in range(QT):
                    recip = obp.tile([128, 1], F32, tag="recip")
                    nc.vector.reciprocal(recip[:], att_ps[qt][:, HID:HID + 1])
                    o_sb = obp.tile([128, HID], F32, tag="osb")
                    nc.scalar.activation(o_sb[:], att_ps[qt][:, 0:HID], AF.Relu,
                                         scale=recip[:, 0:1])
                    nc.sync.dma_start(out=out_d.ap()[qt * 128:(qt + 1) * 128, :], in_=o_sb[:])
    nc.compile()
    return nc


_CACHE = {}


def _programs():
    if "k1" not in _CACHE:
        _CACHE["k1"] = _build_k1()
        _CACHE["k2"] = _build_k2()
    return _CACHE["k1"], _CACHE["k2"]


def _arr(x, dt):
    return np.ascontiguousarray(np.asarray(x), dtype=dt)


def _wrap16(a):
    # flat int list -> [128, n/16] int16: index i at partition i%16, slot
    # i//16, replicated for the 8 Q7 cores
    a = np.asarray(a)
    return np.tile(a.reshape(-1, 16).T, (8, 1)).astype(np.int16)


def _cmajor(adj_blk):
    # [UB, DEG] -> flat vals[i], i = (t*DEG+j)*128 + p, user = t*128+p
    return adj_blk.reshape(NT, 128, DEG).transpose(0, 2, 1).reshape(-1)


def _stack_w(w):
    w4 = w.reshape(DEG, 2, D, HID)
    wa = np.ascontiguousarray(w4[:, 0].reshape(DEG * D, HID))
    wb = np.ascontiguousarray(w4[:, 1].reshape(DEG * D, HID))
    return wa, wb


def _aug_tiled(h):
    import ml_dtypes
    ha = np.concatenate([h, np.ones((NU, 1), np.float32)], axis=1)
    ha = ha.reshape(MT, 128, HID + 1).transpose(1, 0, 2)
    return np.ascontiguousarray(ha.astype(ml_dtypes.bfloat16))


def kernel(review_vecs, user_vecs, item_vecs, user_weights, item_weights,
           adj0, adj1, adj2, adj3, _profile=None):
    rev = _arr(review_vecs, np.float32)
    uv = _arr(user_vecs, np.float32)
    iv = _arr(item_vecs, np.float32)
    wu = _arr(user_weights, np.float32)
    wi = _arr(item_weights, np.float32)
    a0, a1, a2, a3 = (np.asarray(a).astype(np.int64) for a in (adj0, adj1, adj2, adj3))

    wa_u, wb_u = _stack_w(wu)
    wa_i, wb_i = _stack_w(wi)
    import ml_dtypes
    uvt = np.ascontiguousarray(uv.T.astype(ml_dtypes.bfloat16))
    ivt = np.ascontiguousarray(iv.T.astype(ml_dtypes.bfloat16))

    k1, k2 = _programs()
    cores = list(range(N_CORES))

    in_maps1 = []
    for c in cores:
        bidx = np.zeros((2, 128, NG // 16), np.int16)
        iidx = np.zeros((2, 128, NG // 16), np.int16)
        selm = np.zeros((2, 128, NSLOT, BLK), np.float32)
        for side, (a_rev, a_oth) in enumerate(((a0, a1), (a2, a3))):
            rvals = _cmajor(a_rev[c * UB:(c + 1) * UB])
            ovals = _cmajor(a_oth[c * UB:(c + 1) * UB])
            bidx[side] = _wrap16(rvals // BLK)
            iidx[side] = _wrap16(ovals)
            sel = (rvals % BLK).reshape(NSLOT, 128).T  # [p, c]
            for b in range(BLK):
                selm[side, :, :, b] = (sel == b)
        in_maps1.append({
            "rev": rev, "usert": uv, "itemt": iv,
            "wa_u": wa_u, "wb_u": wb_u, "wa_i": wa_i, "wb_i": wb_i,
            "bidx": bidx, "selm": selm, "iidx": iidx,
            "vtu": uvt, "vtuq": np.ascontiguousarray(uvt[:, c * QB:(c + 1) * QB]),
            "vti": ivt, "vtiq": np.ascontiguousarray(ivt[:, c * QB:(c + 1) * QB]),
        })
    r1 = run_bass_kernel_spmd(k1, in_maps1, core_ids=cores, trace=_profile is not None)
    h_user = np.concatenate([r1.results[c]["hu"] for c in cores], axis=0)
    h_item = np.concatenate([r1.results[c]["hi"] for c in cores], axis=0)

    hau = _aug_tiled(h_user)
    hai = _aug_tiled(h_item)
    in_maps2 = [{
        "et": r1.results[c]["et"], "hau": hau, "hai": hai,
    } for c in cores]
    r2 = run_bass_kernel_spmd(k2, in_maps2, core_ids=cores, trace=_profile is not None)

    user_out = np.concatenate([r2.results[c]["uo"] for c in cores], axis=0)
    item_out = np.concatenate([r2.results[c]["io"] for c in cores], axis=0)

    if _profile is not None:
        _profile["k1"] = r1
        _profile["k2"] = r2
    return user_out, item_out


# revision 13
# speedup vs baseline: 2.4800x; 1.4185x over previous
"""Trainium2 Bass kernel for nn_AttentionAggregator (gnn_message_passing).

Strategy (8-way data parallel over nodes, per the sharding hint):
  - Algebraic reorder: relu(softmax(Q K^T) @ C @ W) == relu(softmax(Q K^T) @ (C @ W)),
    shrinking the dominant matmul from [4096,4096]@[4096,2048] to [4096,4096]@[4096,128].
  - The serial cost floor on-device is GPSIMD descriptor generation for the
    data-dependent row gathers (~8 ns/row).  Launch 1 therefore runs the
    gathers on GPSIMD while *all* other engines compute underneath that
    shadow:
      * review rows arrive as 1KB 4-row blocks via the custom dma_gather
        (block id r//4 fits int16 with no windowing/sorting); a 4-way
        masked DVE select picks row r%4 per entry;
      * item/user rows (4096-row tables) gather directly with int16 ids;
      * gathered tiles are PE-transposed in slot pairs and projected against
        host-restacked weight blocks -> h blocks [512,128] per side;
      * concurrently the dense scores S^T = K q^T run on the tensor engine and
        exp(S/8) on ScalarE (scores ~N(0,1): no max subtraction), emitting
        E^T in bf16 to DRAM.
  - Host relays h (tiny) and E^T between launches.
  - Launch 2 (per core): PV matmul (bf16 E^T stationary -> fast weight load)
    against [h | 1] bf16 (fused row-sum), reciprocal-normalize + relu.
"""

import sys

for _p in ("/opt/trn_rl_repo",):
    if _p not in sys.path:
        sys.path.append(_p)

import numpy as np

import concourse.bacc as bacc
import concourse.bass as bass
import concourse.mybir as mybir
import concourse.tile as tile
from concourse.bass_utils import run_bass_kernel_spmd
from concourse.masks import make_identity

F32 = mybir.dt.float32
BF16 = mybir.dt.bfloat16
I16 = mybir.dt.int16
I8 = mybir.dt.int8
AF = mybir.ActivationFunctionType
MULT = mybir.AluOpType.mult
ADD = mybir.AluOpType.add

N_REV, NU, DEG, D, HID = 100000, 4096, 16, 64, 128
N_CORES = 8
UB = NU // N_CORES          # 512 rows per core per side
NT = UB // 128              # 4 user tiles per core
NSLOT = NT * DEG            # 64 gathered slots per side (c = t*16 + j)
NG = UB * DEG               # 8192 gathered rows per table per side
MT = NU // 128              # 32 m tiles
QB = UB
QT = QB // 128
G = 2                       # m-tiles per QK/exp group
BLK = 4                     # review rows per gathered block
SCALE = 1.0 / float(np.sqrt(D))


def _build_k1():
    nc = bacc.Bacc("TRN2", target_bir_lowering=False, debug=False,
                   enable_asserts=True, num_devices=N_CORES,
                   num_swdge_queues=4)
    rev = nc.dram_tensor("rev", [N_REV, D], F32, kind="ExternalInput")
    usert = nc.dram_tensor("usert", [NU, D], F32, kind="ExternalInput")
    itemt = nc.dram_tensor("itemt", [NU, D], F32, kind="ExternalInput")
    wa_u = nc.dram_tensor("wa_u", [DEG * D, HID], F32, kind="ExternalInput")
    wb_u = nc.dram_tensor("wb_u", [DEG * D, HID], F32, kind="ExternalInput")
    wa_i = nc.dram_tensor("wa_i", [DEG * D, HID], F32, kind="ExternalInput")
    wb_i = nc.dram_tensor("wb_i", [DEG * D, HID], F32, kind="ExternalInput")
    bidx = nc.dram_tensor("bidx", [2, 128, NG // 16], I16, kind="ExternalInput")
    selm = nc.dram_tensor("selm", [2, 128, NSLOT, BLK], I8, kind="ExternalInput")
    iidx = nc.dram_tensor("iidx", [2, 128, NG // 16], I16, kind="ExternalInput")
    vtu = nc.dram_tensor("vtu", [D, NU], BF16, kind="ExternalInput")
    vtuq = nc.dram_tensor("vtuq", [D, QB], BF16, kind="ExternalInput")
    vti = nc.dram_tensor("vti", [D, NU], BF16, kind="ExternalInput")
    vtiq = nc.dram_tensor("vtiq", [D, QB], BF16, kind="ExternalInput")
    hu = nc.dram_tensor("hu", [UB, HID], F32, kind="ExternalOutput")
    hi = nc.dram_tensor("hi", [UB, HID], F32, kind="ExternalOutput")
    et = nc.dram_tensor("et", [2, MT, 128, QB], BF16, kind="ExternalOutput")

    with tile.TileContext(nc) as tc:
        with (
            tc.tile_pool(name="singles", bufs=1) as singles,
            tc.tile_pool(name="stgp", bufs=3) as stgp,
            tc.tile_pool(name="xp", bufs=3) as xp,
            tc.tile_pool(name="xtp", bufs=2) as xtp,
            tc.tile_pool(name="outb", bufs=4) as outb,
            tc.tile_pool(name="vtp", bufs=1) as vtp,
            tc.tile_pool(name="etp", bufs=3) as etp,
            tc.tile_pool(name="sps", bufs=2, space="PSUM") as sps,
            tc.tile_pool(name="tps", bufs=2, space="PSUM") as tps,
            tc.tile_pool(name="hps", bufs=2, space="PSUM") as hps,
        ):
            ident = singles.tile([128, 128], F32)
            make_identity(nc, ident[:])
            w_sb = {}
            for nm, t in (("wa_u", wa_u), ("wb_u", wb_u), ("wa_i", wa_i), ("wb_i", wb_i)):
                w = singles.tile([128, DEG // 2, HID], F32, tag=nm, name=f"{nm}_sb")
                nc.sync.dma_start(out=w[:], in_=t.ap().rearrange("(q k) n -> k q n", q=DEG // 2))
                w_sb[nm] = w
            bidx_sb = singles.tile([128, 2, NG // 16], I16)
            nc.sync.dma_start(out=bidx_sb[:], in_=bidx.ap().rearrange("a p s -> p a s"))
            iidx_sb = singles.tile([128, 2, NG // 16], I16)
            nc.sync.dma_start(out=iidx_sb[:], in_=iidx.ap().rearrange("a p s -> p a s"))
            selm_sb = singles.tile([128, 2, NSLOT, BLK], I8)
            nc.sync.dma_start(out=selm_sb[:], in_=selm.ap().rearrange("a p c b -> p a c b"))

            for side, (itbl, vt_d, vtq_d, wa, wb, hout) in enumerate((
                (itemt, vtu, vtuq, "wa_u", "wb_u", hu),
                (usert, vti, vtiq, "wa_i", "wb_i", hi),
            )):
                # ---- dense scores + exp (runs under the gather shadow) ----
                vt_sb = vtp.tile([D, NU], BF16, tag="vt")
                vtq_sb = vtp.tile([D, QB], BF16, tag="vtq")
                nc.sync.dma_start(out=vt_sb[:], in_=vt_d.ap())
                nc.sync.dma_start(out=vtq_sb[:], in_=vtq_d.ap())
                for g in range(MT // G):
                    s_ps = sps.tile([128, G, QB], F32, tag="sps")
                    for k in range(G):
                        m = g * G + k
                        nc.tensor.matmul(
                            s_ps[:, k, :],
                            lhsT=vt_sb[:, m * 128:(m + 1) * 128],
                            rhs=vtq_sb[:],
                            start=True, stop=True,
                        )
                    etb = etp.tile([128, G, QB], BF16, tag="etb")
                    nc.scalar.activation(etb[:], s_ps[:], AF.Exp, scale=SCALE)
                    nc.sync.dma_start(
                        out=et.ap()[side, g * G:(g + 1) * G, :, :].rearrange("g p q -> p g q"),
                        in_=etb[:],
                    )

                # ---- per-tile gathers rotated across the 4 SWDGE queues;
                # select + transpose + project pipelined right behind each tile
                for t in range(NT):
                    sl = slice(t * DEG, (t + 1) * DEG)
                    iw = slice(t * (NG // NT // 16), (t + 1) * (NG // NT // 16))
                    TG = NG // NT    # 2048 rows per tile
                    q0 = (side * NT + t) % 4
                    stg = stgp.tile([128, DEG, BLK * D], F32, tag="stg")
                    nc.gpsimd.dma_gather(
                        out_ap=stg[:],
                        in_ap=rev.ap().rearrange("(n b) e -> n (b e)", b=BLK),
                        idxs_ap=bidx_sb[:, side, iw],
                        num_idxs=TG, num_idxs_reg=TG, elem_size=BLK * D,
                        single_packet=False, queue_num=q0,
                    )
                    XI = xp.tile([128, DEG, D], F32, tag="XI")
                    nc.gpsimd.dma_gather(
                        out_ap=XI[:], in_ap=itbl.ap(),
                        idxs_ap=iidx_sb[:, side, iw],
                        num_idxs=TG, num_idxs_reg=TG, elem_size=D,
                        single_packet=False, queue_num=(q0 + 1) % 4,
                    )
                    XR = xp.tile([128, DEG, D], F32, tag="XR")
                    nc.vector.tensor_copy(XR[:], stg[:, :, 0:D])
                    for b in range(1, BLK):
                        mb = selm_sb[:, side, sl, b][:, :, None].broadcast_to([128, DEG, D])
                        nc.vector.copy_predicated(XR[:], mb, stg[:, :, b * D:(b + 1) * D])

                    XT = xtp.tile([128, DEG, 128], F32, tag="XT")
                    for q in range(DEG // 2):
                        ps_r = tps.tile([128, 128], F32, tag="tps", name=f"tr{side}_{t}_{q}")
                        nc.tensor.transpose(ps_r[:], XR[:, 2 * q: 2 * q + 2, :], ident[:])
                        if q % 2 == 0:
                            nc.vector.tensor_copy(XT[:, q, :], ps_r[:])
                        else:
                            nc.scalar.activation(XT[:, q, :], ps_r[:], AF.Copy)
                        ps_i = tps.tile([128, 128], F32, tag="tps", name=f"ti{side}_{t}_{q}")
                        nc.tensor.transpose(ps_i[:], XI[:, 2 * q: 2 * q + 2, :], ident[:])
                        if q % 2 == 0:
                            nc.scalar.activation(XT[:, DEG // 2 + q, :], ps_i[:], AF.Copy)
                        else:
                            nc.vector.tensor_copy(XT[:, DEG // 2 + q, :], ps_i[:])
                    h_ps = hps.tile([128, HID], F32, tag="hps")
                    for q in range(DEG // 2):
                        nc.tensor.matmul(h_ps[:], lhsT=XT[:, q, :], rhs=w_sb[wa][:, q, :],
                                         start=(q == 0), stop=False, skip_group_check=True)
                        nc.tensor.matmul(h_ps[:], lhsT=XT[:, DEG // 2 + q, :], rhs=w_sb[wb][:, q, :],
                                         start=False, stop=(q == DEG // 2 - 1), skip_group_check=True)
                    h_sb = outb.tile([128, HID], F32, tag="hsb")
                    nc.vector.tensor_copy(h_sb[:], h_ps[:])
                    nc.sync.dma_start(out=hout.ap()[t * 128:(t + 1) * 128, :], in_=h_sb[:])
    nc.compile()
    return nc


def _build_k2():
    nc = bacc.Bacc("TRN2", target_bir_lowering=False, debug=False,
                   enable_asserts=True, num_devices=N_CORES)
    et = nc.dram_tensor("et", [2, MT, 128, QB], BF16, kind="ExternalInput")
    hau = nc.dram_tensor("hau", [128, MT, HID + 1], BF16, kind="ExternalInput")
    hai = nc.dram_tensor("hai", [128, MT, HID + 1], BF16, kind="ExternalInput")
    uo = nc.dram_tensor("uo", [QB, HID], F32, kind="ExternalOutput")
    io = nc.dram_tensor("io", [QB, HID], F32, kind="ExternalOutput")

    with tile.TileContext(nc) as tc:
        with (
            tc.tile_pool(name="etp", bufs=2) as etp,
            tc.tile_pool(name="ha", bufs=2) as hap,
            tc.tile_pool(name="ob", bufs=4) as obp,
            tc.tile_pool(name="aps", bufs=1, space="PSUM") as aps,
        ):
            for side, (ha_d, out_d) in enumerate(((hau, uo), (hai, io))):
                et_sb = etp.tile([128, MT, QB], BF16, tag="et")
                nc.sync.dma_start(out=et_sb[:], in_=et.ap()[side].rearrange("m p q -> p m q"))
                ha_sb = hap.tile([128, MT, HID + 1], BF16, tag="ha")
                nc.sync.dma_start(out=ha_sb[:], in_=ha_d.ap())

                att_ps = [aps.tile([128, HID + 1], F32, tag=f"att{qt}", name=f"att{qt}_{side}")
                          for qt in range(QT)]
                for m in range(MT):
                    for qt in range(QT):
                        nc.tensor.matmul(
                            att_ps[qt][:],
                            lhsT=et_sb[:, m, qt * 128:(qt + 1) * 128],
                            rhs=ha_sb[:, m, :],
                            start=(m == 0), stop=(m == MT - 1),
                            skip_group_check=True,
                        )
                for qt in range(QT):
                    recip = obp.tile([128, 1], F32, tag="recip")
                    nc.vector.reciprocal(recip[:], att_ps[qt][:, HID:HID + 1])
                    o_sb = obp.tile([128, HID], F32, tag="osb")
                    nc.scalar.activation(o_sb[:], att_ps[qt][:, 0:HID], AF.Relu,
                                         scale=recip[:, 0:1])
                    nc.sync.dma_start(out=out_d.ap()[qt * 128:(qt + 1) * 128, :], in_=o_sb[:])
    nc.compile()
    return nc


_CACHE = {}


def _programs():
    if "k1" not in _CACHE:
        _CACHE["k1"] = _build_k1()
        _CACHE["k2"] = _build_k2()
    return _CACHE["k1"], _CACHE["k2"]


def _arr(x, dt):
    return np.ascontiguousarray(np.asarray(x), dtype=dt)


def _wrap16(a):
    # flat int list -> [128, n/16] int16: index i at partition i%16, slot
    # i//16, replicated for the 8 Q7 cores
    a = np.asarray(a)
    return np.tile(a.reshape(-1, 16).T, (8, 1)).astype(np.int16)


def _cmajor(adj_blk):
    # [UB, DEG] -> flat vals[i], i = (t*DEG+j)*128 + p, user = t*128+p
    return adj_blk.reshape(NT, 128, DEG).transpose(0, 2, 1).reshape(-1)


def _stack_w(w):
    w4 = w.reshape(DEG, 2, D, HID)
    wa = np.ascontiguousarray(w4[:, 0].reshape(DEG * D, HID))
    wb = np.ascontiguousarray(w4[:, 1].reshape(DEG * D, HID))
    return wa, wb


def _aug_tiled(h):
    import ml_dtypes
    ha = np.concatenate([h, np.ones((NU, 1), np.float32)], axis=1)
    ha = ha.reshape(MT, 128, HID + 1).transpose(1, 0, 2)
    return np.ascontiguousarray(ha.astype(ml_dtypes.bfloat16))


def kernel(review_vecs, user_vecs, item_vecs, user_weights, item_weights,
           adj0, adj1, adj2, adj3, _profile=None):
    rev = _arr(review_vecs, np.float32)
    uv = _arr(user_vecs, np.float32)
    iv = _arr(item_vecs, np.float32)
    wu = _arr(user_weights, np.float32)
    wi = _arr(item_weights, np.float32)
    a0, a1, a2, a3 = (np.asarray(a).astype(np.int64) for a in (adj0, adj1, adj2, adj3))

    wa_u, wb_u = _stack_w(wu)
    wa_i, wb_i = _stack_w(wi)
    import ml_dtypes
    uvt = np.ascontiguousarray(uv.T.astype(ml_dtypes.bfloat16))
    ivt = np.ascontiguousarray(iv.T.astype(ml_dtypes.bfloat16))

    k1, k2 = _programs()
    cores = list(range(N_CORES))

    in_maps1 = []
    for c in cores:
        bidx = np.zeros((2, 128, NG // 16), np.int16)
        iidx = np.zeros((2, 128, NG // 16), np.int16)
        selm = np.zeros((2, 128, NSLOT, BLK), np.int8)
        for side, (a_rev, a_oth) in enumerate(((a0, a1), (a2, a3))):
            rvals = _cmajor(a_rev[c * UB:(c + 1) * UB])
            ovals = _cmajor(a_oth[c * UB:(c + 1) * UB])
            bidx[side] = _wrap16(rvals // BLK)
            iidx[side] = _wrap16(ovals)
            sel = (rvals % BLK).reshape(NSLOT, 128).T  # [p, c]
            for b in range(BLK):
                selm[side, :, :, b] = (sel == b)
        in_maps1.append({
            "rev": rev, "usert": uv, "itemt": iv,
            "wa_u": wa_u, "wb_u": wb_u, "wa_i": wa_i, "wb_i": wb_i,
            "bidx": bidx, "selm": selm, "iidx": iidx,
            "vtu": uvt, "vtuq": np.ascontiguousarray(uvt[:, c * QB:(c + 1) * QB]),
            "vti": ivt, "vtiq": np.ascontiguousarray(ivt[:, c * QB:(c + 1) * QB]),
        })
    r1 = run_bass_kernel_spmd(k1, in_maps1, core_ids=cores, trace=_profile is not None)
    h_user = np.concatenate([r1.results[c]["hu"] for c in cores], axis=0)
    h_item = np.concatenate([r1.results[c]["hi"] for c in cores], axis=0)

    hau = _aug_tiled(h_user)
    hai = _aug_tiled(h_item)
    in_maps2 = [{
        "et": r1.results[c]["et"], "hau": hau, "hai": hai,
    } for c in cores]
    r2 = run_bass_kernel_spmd(k2, in_maps2, core_ids=cores, trace=_profile is not None)

    user_out = np.concatenate([r2.results[c]["uo"] for c in cores], axis=0)
    item_out = np.concatenate([r2.results[c]["io"] for c in cores], axis=0)

    if _profile is not None:
        _profile["k1"] = r1
        _profile["k2"] = r2
    return user_out, item_out
